# revision 1
# baseline (speedup 1.0000x reference)
"""Decagon-style 2-type/4-relation GNN message passing on 8 Trainium2 NeuronCores.

Strategy (graph/data parallel, per the sharding hint):
  - Nodes of both types are row-sharded across the 8 cores (6250 rows each).
  - Each layer, every core projects its own h-shard through the 4 relation
    weight matrices (fp16), and the projected message tables are AllGathered
    into per-core DRAM tables [50176, 64] fp16.
  - Edges are dst-sharded: core c owns all edges whose dst lands in its shard.
    Host groups each core's edges by 32-node dst chunk, pads each chunk's edge
    list to whole 128-edge tiles (shared tile counts across cores = SPMD).
  - Per 128-node output group: indirect-DMA gather of the src rows of the
    message table (fp16, 128B rows), a one-hot "segment matrix" S
    (S[p, c] = ew[p] if dst_local[p] == c else 0, built on DVE), and PE
    matmuls psum[32c:32c+32, :] += S_tile.T @ msg_tile accumulate the
    weighted segment sums for 4 chunks at a time in fp32 PSUM.
  - ReLU drain feeds (a) fp32 outputs for layers 1/2/5, (b) an fp16
    PE-transpose into a feature-major copy used for next layer's projections.
"""

import sys

sys.path.insert(0, "/opt/trn_rl_repo")

import numpy as np

N_NODES = 50000
F_IN = 128
H = 64
NET = 4
N_CORES = 8
CHUNK = 32  # dst nodes per one-hot matmul column block (psum col group)
GRP = 128  # dst nodes per psum group (4 chunks)
P = 128


def _ceil(a, b):
    return -(-a // b)


def _prep(src, dst, ew, n_nodes, ns, ns_pad):
    """Per-relation, per-core edge grouping. Returns per-relation dict with
    shared (SPMD) tile structure and per-core [128, T] slot grids."""
    nchunk = ns_pad // CHUNK
    rels = []
    for k in range(src.shape[0]):
        per_core = []
        tiles_per_chunk = np.ones(nchunk, np.int64)
        for c in range(N_CORES):
            m = (dst[k] >= c * ns) & (dst[k] < (c + 1) * ns)
            dl = (dst[k][m] - c * ns).astype(np.int64)
            s = src[k][m].astype(np.int64)
            w = ew[k][m]
            ch = dl // CHUNK
            order = np.argsort(ch, kind="stable")
            dl, s, w, ch = dl[order], s[order], w[order], ch[order]
            cnt = np.bincount(ch, minlength=nchunk)
            tiles_per_chunk = np.maximum(tiles_per_chunk, _ceil(cnt, P))
            per_core.append((dl, s, w, ch, cnt))
        tile_base = np.concatenate(([0], np.cumsum(tiles_per_chunk)))
        T = int(tile_base[-1])
        gidx = np.zeros((N_CORES, P, T), np.int32)
        dlv = np.zeros((N_CORES, P, T), np.float16)
        eww = np.zeros((N_CORES, P, T), np.float16)
        for c in range(N_CORES):
            dl, s, w, ch, cnt = per_core[c]
            idx0 = np.concatenate(([0], np.cumsum(cnt)))
            rank = np.arange(len(ch)) - idx0[ch]
            slots = tile_base[ch] * P + rank
            g = np.zeros(T * P, np.int32)
            d = np.zeros(T * P, np.float16)
            e = np.zeros(T * P, np.float16)
            g[slots] = ((s // ns) * ns_pad + (s % ns)).astype(np.int32)
            d[slots] = (dl % CHUNK).astype(np.float16)
            e[slots] = w.astype(np.float16)
            # slot i -> (partition i % 128, tile i // 128)
            gidx[c] = g.reshape(T, P).T
            dlv[c] = d.reshape(T, P).T
            eww[c] = e.reshape(T, P).T
        rels.append(
            dict(T=T, tile_base=tile_base, tiles_per_chunk=tiles_per_chunk,
                 gidx=gidx, dl=dlv, ew=eww)
        )
    return rels


def _build(rels, ns_pad, n_layers=5, out_layers=(0, 1, 4)):
    import concourse.bass as bass
    import concourse.mybir as mybir
    import concourse.tile as tile
    from concourse import bacc
    from concourse.masks import make_identity

    F16 = mybir.dt.float16
    F32 = mybir.dt.float32
    AF = mybir.ActivationFunctionType
    OP = mybir.AluOpType

    NG = ns_pad // GRP
    nchunk = ns_pad // CHUNK
    NROWS = N_CORES * ns_pad
    # tiles per (rel, group)
    gtiles = [
        [int(r["tile_base"][4 * (g + 1)] - r["tile_base"][4 * g]) for g in range(NG)]
        for r in rels
    ]
    MAXGT = max(max(gt) for gt in gtiles)

    nc = bacc.Bacc("TRN2", target_bir_lowering=False, debug=False,
                   num_devices=N_CORES)

    xT = [nc.dram_tensor(f"x{j}T", [F_IN, ns_pad], F16, kind="ExternalInput")
          for j in range(2)]
    w1 = nc.dram_tensor("w1", [NET * F_IN, H], F16, kind="ExternalInput")
    wl = (nc.dram_tensor("wl", [(n_layers - 1) * NET * H, H], F16,
                         kind="ExternalInput") if n_layers > 1 else None)
    gidx_d = [nc.dram_tensor(f"gidx{k}", [P, rels[k]["T"]], mybir.dt.int32,
                             kind="ExternalInput") for k in range(NET)]
    dl_d = [nc.dram_tensor(f"dl{k}", [P, rels[k]["T"]], F16,
                           kind="ExternalInput") for k in range(NET)]
    ew_d = [nc.dram_tensor(f"ew{k}", [P, rels[k]["T"]], F16,
                           kind="ExternalInput") for k in range(NET)]
    outs = [nc.dram_tensor(f"out{t}", [len(out_layers) * P, NG * H], F32,
                           kind="ExternalOutput") for t in range(2)]

    cc_in = [nc.dram_tensor(f"ccin{k}", [ns_pad, H], F16) for k in range(NET)]
    tbl = [nc.dram_tensor(f"tbl{k}", [NROWS, H], F16, addr_space="Shared")
           for k in range(NET)]

    with tile.TileContext(nc) as tc:
        with (
            tc.tile_pool(name="res", bufs=1) as res,
            tc.tile_pool(name="msg", bufs=3) as msgp,
            tc.tile_pool(name="sp", bufs=3) as sp,
            tc.tile_pool(name="small", bufs=3) as small,
            tc.tile_pool(name="pagg", bufs=3, space="PSUM") as pagg,
            tc.tile_pool(name="ptr", bufs=2, space="PSUM") as ptr,
            tc.tile_pool(name="ppr", bufs=2, space="PSUM") as ppr,
        ):
            # ---- resident loads ----
            gidx_sb = [res.tile([P, rels[k]["T"]], mybir.dt.int32, tag=f"gi{k}", name=f"gi{k}")
                       for k in range(NET)]
            dl_sb = [res.tile([P, rels[k]["T"]], F16, tag=f"dl{k}", name=f"dlsb{k}")
                     for k in range(NET)]
            ew_sb = [res.tile([P, rels[k]["T"]], F16, tag=f"ew{k}", name=f"ewsb{k}")
                     for k in range(NET)]
            for k in range(NET):
                nc.sync.dma_start(gidx_sb[k][:], gidx_d[k][:])
                nc.sync.dma_start(dl_sb[k][:], dl_d[k][:])
                nc.sync.dma_start(ew_sb[k][:], ew_d[k][:])
            xT_sb = [res.tile([F_IN, ns_pad], F16, tag=f"xT{j}", name=f"xTsb{j}") for j in range(2)]
            for j in range(2):
                nc.sync.dma_start(xT_sb[j][:], xT[j][:])
            w1_sb = res.tile([F_IN, NET * H], F16, tag="w1", name="w1sb")
            nc.sync.dma_start(
                w1_sb[:].rearrange("p (k f) -> p k f", k=NET),
                w1.ap().rearrange("(k p) f -> p k f", k=NET),
            )
            if n_layers > 1:
                nwl = (n_layers - 1) * NET
                wl_sb = res.tile([H, nwl * H], F16, tag="wl", name="wlsb")
                nc.sync.dma_start(
                    wl_sb[:].rearrange("p (m f) -> p m f", m=nwl),
                    wl.ap().rearrange("(m p) f -> p m f", m=nwl),
                )
            ident = res.tile([P, P], F16, tag="ident", name="ident")
            make_identity(nc, ident[:])
            iota_i = res.tile([P, MAXGT * CHUNK], mybir.dt.int16, tag="ioi", name="iotai")
            nc.gpsimd.iota(
                iota_i[:].rearrange("p (t c) -> p t c", c=CHUNK),
                pattern=[[0, MAXGT], [1, CHUNK]], base=0, channel_multiplier=0,
            )
            iota_f = res.tile([P, MAXGT * CHUNK], F16, tag="iof", name="iotaf")
            nc.vector.tensor_copy(out=iota_f[:], in_=iota_i[:])
            hT = [res.tile([H, ns_pad], F16, tag=f"hT{j}", name=f"hTsb{j}") for j in range(2)]
            f32st = res.tile([P, NG * H], F32, tag="f32st", name="f32st")

            # ---- layer-1 projections from xT ----
            for g in range(NG):
                pp = ppr.tile([P, NET * H], F32, tag="pp", name="pp", padded_shape=[P, 512])
                for k in range(NET):
                    nc.tensor.matmul(
                        out=pp[:, k * H:(k + 1) * H],
                        lhsT=xT_sb[k % 2][:, g * GRP:(g + 1) * GRP],
                        rhs=w1_sb[:, k * H:(k + 1) * H],
                        start=True, stop=True,
                    )
                tmp = small.tile([P, NET * H], F16, tag="projdr", name="projdr")
                nc.scalar.activation(out=tmp[:], in_=pp[:], func=AF.Copy)
                for k in range(NET):
                    nc.sync.dma_start(
                        cc_in[k].ap().rearrange("(t p) f -> p t f", p=P)[:, g, :],
                        tmp[:, k * H:(k + 1) * H],
                    )

            # ---- layers ----
            for L in range(n_layers):
                relu = L < n_layers - 1
                for k in range(NET):
                    nc.gpsimd.collective_compute(
                        "AllGather", OP.bypass,
                        replica_groups=[list(range(N_CORES))],
                        ins=[cc_in[k].ap().opt()], outs=[tbl[k].ap().opt()],
                    )
                for it in range(2):
                    rels_it = (2 * it, 2 * it + 1)
                    for g in range(NG):
                        mbufs, sbufs = {}, {}
                        for k in rels_it:
                            b0 = int(rels[k]["tile_base"][4 * g])
                            n_t = gtiles[k][g]
                            mb = msgp.tile([P, MAXGT * H], F16, tag=f"m{k % 2}", name=f"mb{k % 2}")
                            # HW indirect DMA consumes one offset per
                            # partition-descriptor; gather one 128-row tile
                            # per op with a packed [P,1] offset column.
                            for t in range(n_t):
                                ipk = small.tile([P, 1], mybir.dt.int32,
                                                 tag=f"ipk{k % 2}",
                                                 name=f"ipk{k % 2}")
                                nc.vector.tensor_copy(
                                    out=ipk[:],
                                    in_=gidx_sb[k][:, b0 + t:b0 + t + 1])
                                nc.gpsimd.indirect_dma_start(
                                    out=mb[:, t * H:(t + 1) * H],
                                    out_offset=None,
                                    in_=tbl[k][:],
                                    in_offset=bass.IndirectOffsetOnAxis(
                                        ap=ipk[:], axis=0),
                                )
                            oh = sp.tile([P, MAXGT * CHUNK], F16, tag=f"oh{k % 2}", name=f"ohb{k % 2}")
                            sb = sp.tile([P, MAXGT * CHUNK], F16, tag=f"S{k % 2}", name=f"Sb{k % 2}")
                            dlb = dl_sb[k][:, b0:b0 + n_t].to_broadcast(
                                [P, n_t, CHUNK])
                            ewb = ew_sb[k][:, b0:b0 + n_t].to_broadcast(
                                [P, n_t, CHUNK])
                            i3 = iota_f[:, :n_t * CHUNK].rearrange(
                                "p (t c) -> p t c", c=CHUNK)
                            nc.vector.tensor_tensor(
                                out=oh[:, :n_t * CHUNK].rearrange(
                                    "p (t c) -> p t c", c=CHUNK),
                                in0=i3, in1=dlb, op=OP.is_equal)
                            nc.vector.tensor_tensor(
                                out=sb[:, :n_t * CHUNK].rearrange(
                                    "p (t c) -> p t c", c=CHUNK),
                                in0=oh[:, :n_t * CHUNK].rearrange(
                                    "p (t c) -> p t c", c=CHUNK),
                                in1=ewb, op=OP.mult)
                            mbufs[k], sbufs[k] = mb, sb
                        # matmul schedule for this group's psum
                        sched = []  # (k, local_tile, ch4) — ch4-major so only
                        # one psum accumulation group is open at a time
                        for ch4 in range(4):
                            for k in rels_it:
                                tb = rels[k]["tile_base"]
                                b0 = int(tb[4 * g])
                                for t in range(int(tb[4 * g + ch4]) - b0,
                                               int(tb[4 * g + ch4 + 1]) - b0):
                                    sched.append((k, t, ch4))
                        first = [True] * 4
                        last_idx = {}
                        for i, (_, _, ch4) in enumerate(sched):
                            last_idx[ch4] = i
                        pt = pagg.tile([P, H], F32, tag="agg", name="pagt", padded_shape=[P, 512])
                        for i, (k, t, ch4) in enumerate(sched):
                            nc.tensor.matmul(
                                out=pt[ch4 * CHUNK:(ch4 + 1) * CHUNK, :],
                                lhsT=sbufs[k][:, t * CHUNK:(t + 1) * CHUNK],
                                rhs=mbufs[k][:, t * H:(t + 1) * H],
                                start=first[ch4], stop=(last_idx[ch4] == i),
                                tile_position=(0, ch4 * CHUNK),
                            )
                            first[ch4] = False
                        # drains
                        if L in out_layers:
                            if relu:
                                nc.vector.tensor_scalar_max(
                                    f32st[:, g * H:(g + 1) * H], pt[:], 0.0)
                            else:
                                nc.vector.tensor_copy(
                                    out=f32st[:, g * H:(g + 1) * H], in_=pt[:])
                        if L < n_layers - 1:
                            hr = small.tile([P, H], F16, tag="hr", name="hr")
                            nc.scalar.activation(out=hr[:], in_=pt[:],
                                                 func=AF.Relu)
                            pt2 = ptr.tile([H, P], F16, tag="tr", name="ptt", padded_shape=[H, 1024])
                            nc.tensor.matmul(out=pt2[:], lhsT=hr[:], rhs=ident[:],
                                             is_transpose=True, start=True,
                                             stop=True)
                            nc.vector.tensor_copy(
                                out=hT[it][:, g * GRP:(g + 1) * GRP],
                                in_=pt2[:])
                    if L in out_layers:
                        sec = out_layers.index(L)
                        nc.sync.dma_start(
                            outs[it][sec * P:(sec + 1) * P, :], f32st[:])
                # next-layer projections
                if L < n_layers - 1:
                    for g in range(NG):
                        pp = ppr.tile([P, NET * H], F32, tag="pp", name="pp", padded_shape=[P, 512])
                        for k in range(NET):
                            m = L * NET + k
                            nc.tensor.matmul(
                                out=pp[:, k * H:(k + 1) * H],
                                lhsT=hT[k % 2][:, g * GRP:(g + 1) * GRP],
                                rhs=wl_sb[:, m * H:(m + 1) * H],
                                start=True, stop=True,
                            )
                        tmp = small.tile([P, NET * H], F16, tag="projdr", name="projdr")
                        nc.scalar.activation(out=tmp[:], in_=pp[:], func=AF.Copy)
                        for k in range(NET):
                            nc.sync.dma_start(
                                cc_in[k].ap().rearrange(
                                    "(t p) f -> p t f", p=P)[:, g, :],
                                tmp[:, k * H:(k + 1) * H],
                            )
    nc.compile()
    return nc


def _host_inputs(x0, x1, W1, Wl, rels, ns, ns_pad, n_layers=5):  # noqa
    """Per-core input dicts."""
    xs = [np.asarray(x0), np.asarray(x1)]
    in_maps = []
    for c in range(N_CORES):
        m = {}
        for j in range(2):
            sh = np.zeros((F_IN, ns_pad), np.float16)
            sh[:, :ns] = xs[j][c * ns:(c + 1) * ns].T.astype(np.float16)
            m[f"x{j}T"] = sh
        m["w1"] = np.asarray(W1).reshape(NET * F_IN, H).astype(np.float16)
        if n_layers > 1:
            m["wl"] = (np.asarray(Wl)[: n_layers - 1]
                       .reshape((n_layers - 1) * NET * H, H).astype(np.float16))
        for k in range(NET):
            m[f"gidx{k}"] = rels[k]["gidx"][c]
            m[f"dl{k}"] = rels[k]["dl"][c]
            m[f"ew{k}"] = rels[k]["ew"][c]
        in_maps.append(m)
    return in_maps


def _assemble(results, ns, ns_pad, n_out=3):
    NG = ns_pad // GRP
    out = np.zeros((2, N_CORES * ns, n_out * H), np.float32)
    for t in range(2):
        for c in range(N_CORES):
            arr = results[c][f"out{t}"]  # [n_out*128, NG*H]
            for s in range(n_out):
                a = (arr[s * P:(s + 1) * P]
                     .reshape(P, NG, H).transpose(1, 0, 2).reshape(NG * P, H))
                out[t, c * ns:(c + 1) * ns, s * H:(s + 1) * H] = a[:ns]
    return out


def kernel(x0, x1, src, dst, ew, W1, Wl):
    from concourse.bass_utils import run_bass_kernel_spmd

    x0 = np.asarray(x0); x1 = np.asarray(x1)
    src = np.asarray(src); dst = np.asarray(dst); ew = np.asarray(ew)
    W1 = np.asarray(W1); Wl = np.asarray(Wl)

    ns = x0.shape[0] // N_CORES
    ns_pad = _ceil(ns, GRP) * GRP
    rels = _prep(src, dst, ew, x0.shape[0], ns, ns_pad)
    nc = _build(rels, ns_pad)
    in_maps = _host_inputs(x0, x1, W1, Wl, rels, ns, ns_pad)
    global _last
    _last = (nc, in_maps, ns, ns_pad)
    res = run_bass_kernel_spmd(nc, in_maps, core_ids=list(range(N_CORES)))
    return _assemble(res.results, ns, ns_pad)

